# revision 1
# baseline (speedup 1.0000x reference)
"""Trainium2 Bass kernel: per-(image, channel) class-mean replacement (segment mean + gather).

Input:  img [8, 128, 256, 256] f32, gt [8, 1, 256, 256] int32 (labels in [0, 21))
Output: out[b, c, h, w] = mean over pixels p of img[b, c, p] where gt[b, p] == gt[b, h, w]

Sharding: data-parallel over batch — each of the 8 NeuronCores processes one image.

Per-core algorithm (C=128 channels on partitions, HW=65536 pixels on free axis):
  Phase 1 (sums):   PE-transpose 128x128 img chunks (f32r stationary, 1.5 cyc/row) to
                    [pix, ch] PSUM; one Act copy per 1024-px tile PSUM->SBUF (bf16);
                    one-hot [128pix, 32cls] bf16 on DVE from gtT; matmul-accumulate
                    sums[32, 128] and counts[32, 1] into a memset-initialized PSUM bank
                    (all matmuls start=False so the two accumulators share one bank).
                    Emission is software-pipelined two tiles deep so the PE never waits
                    on the PSUM->SBUF copy latency.
  Means:            counts only depend on gt, and the one-hots for the last two tiles
                    are emitted early, so rcp = 1/(cnt+eps) is ready before the last
                    img tile lands; means = sums * rcp (bf16) is the only barrier.
  Phase 2 (gather): PE-transpose stash one-hots to [32cls, pix], DVE copy to SBUF,
                    matmul means.T @ onehotT -> out[128ch, pix] PSUM, Act copy to SBUF,
                    DMA out in 1024-px tiles (software-pipelined one tile deep).
  Boundary:         input ends with 640+384-px DMAs processed as short chains on
                    disjoint engines/tiles; ohT tiles for the first 3 output kilopixels
                    are prebuilt during phase 1; first outputs are 256/256/512 px so the
                    output DMA stream restarts quickly after the means barrier.
"""

import os
import sys

for _p in ("/opt/trn_rl_repo", "/root/.axon_site/_ro/trn_rl_repo"):
    if os.path.isdir(_p) and _p not in sys.path:
        sys.path.append(_p)

import numpy as np

P = 128          # channels == SBUF partitions
HW = 256 * 256   # pixels per image
NCLS = 21
CPAD = 32        # padded class count
CH = 128         # pixels per matmul chunk
NCH = HW // CH   # 512 chunks
TB = 1024        # pixels per input tile / rhs copy group
NT = HW // TB    # 64 tiles
CPT = TB // CH   # 8 chunks per tile
EPS = 1e-8
N_CORES = 8

# chunks whose one-hot + count matmuls are emitted early (the last two tiles),
# so the count accumulation chain finishes before the last img DMA lands
EARLY_CHUNKS = list(range((NT - 2) * CPT, NCH))

_CACHE = {}


def _build_module(variant="full"):
    import concourse.bacc as bacc
    import concourse.mybir as mybir
    import concourse.tile as tile
    from concourse.masks import make_identity

    f32 = mybir.dt.float32
    f32r = mybir.dt.float32r
    bf16 = mybir.dt.bfloat16
    i32 = mybir.dt.int32
    EQ = mybir.AluOpType.is_equal
    MULT = mybir.AluOpType.mult

    nc = bacc.Bacc("TRN2", target_bir_lowering=False, debug=False)
    img = nc.dram_tensor("img", [P, HW], f32r, kind="ExternalInput")
    gt = nc.dram_tensor("gt", [HW], i32, kind="ExternalInput")
    out = nc.dram_tensor("out", [P, HW], f32, kind="ExternalOutput")

    def gtcol(gc):
        # chunk gc's labels live in gtT at col 32*(gc%16) + gc//16
        return 32 * (gc % 16) + gc // 16

    with tile.TileContext(nc) as tc:
        with (
            tc.tile_pool(name="constp", bufs=1) as constp,
            tc.tile_pool(name="imgp", bufs=6) as imgp,
            tc.tile_pool(name="rhsp", bufs=6) as rhsp,
            tc.tile_pool(name="stashp", bufs=1) as stashp,
            tc.tile_pool(name="ohspre", bufs=1) as ohspre,
            tc.tile_pool(name="ohsp", bufs=4) as ohsp,
            tc.tile_pool(name="outp", bufs=5) as outp,
            tc.tile_pool(name="psS", bufs=1, space="PSUM") as psS,
        ):
            # ---- constants ----
            ident32 = constp.tile([P, P], f32, tag="id32")
            make_identity(nc, ident32[:])
            ident16 = constp.tile([P, P], bf16, tag="id16")
            nc.vector.tensor_copy(out=ident16[:], in_=ident32[:])
            identr = constp.tile([P, P], f32r, tag="idr")
            nc.vector.tensor_copy(out=identr[:], in_=ident32[:])
            iota_b = constp.tile([P, CPAD], bf16, tag="iota")
            nc.gpsimd.iota(
                iota_b[:],
                pattern=[[1, CPAD]],
                base=0,
                channel_multiplier=0,
                allow_small_or_imprecise_dtypes=True,
            )
            ones1 = constp.tile([P, 1], bf16, tag="ones1")
            nc.vector.memset(ones1[:], 1.0)

            # ---- gt preprocessing: labels for chunk gc onto partitions ----
            gtn_i = constp.tile([32, HW // 32], i32, tag="gtn_i")
            nc.sync.dma_start(
                out=gtn_i[:], in_=gt.ap().rearrange("(p f) -> p f", p=32)
            )
            gtn_f = constp.tile([32, HW // 32], f32, tag="gtn_f")
            for h in range(2):
                nc.vector.tensor_copy(
                    out=gtn_f[:, h * 1024 : (h + 1) * 1024],
                    in_=gtn_i[:, h * 1024 : (h + 1) * 1024],
                )
            gtT = constp.tile([P, NCH], f32, tag="gtT")
            with tc.tile_pool(name="psGT", bufs=1, space="PSUM") as psGT:
                gps = psGT.tile([P, NCH], f32, tag="gt")
                for b in range(16):
                    nc.tensor.transpose(
                        out=gps[:, b * 32 : (b + 1) * 32],
                        in_=gtn_f[:, b * P : (b + 1) * P],
                        identity=ident32[0:32, 0:32],
                    )
                nc.vector.tensor_copy(out=gtT[:], in_=gps[:])

            # one-hot stash for the whole image: chunk gc at cols [32gc, 32gc+32)
            stash = stashp.tile([P, CPAD * NCH], bf16, tag="stash")
            sums_cnt = psS.tile([CPAD, P + 4], f32, tag="sums")
            nc.vector.memset(sums_cnt[:], 0.0)
            sums = sums_cnt[:, 0:P]
            cnt1 = sums_cnt[:, P : P + 1]
            means = constp.tile([CPAD, P], bf16, tag="means")
            rcp = constp.tile([CPAD, 1], f32, tag="rcp")
            cnt_eps = constp.tile([CPAD, 1], f32, tag="cnt_eps")

            def emit_onehot(gc):
                oh = stash[:, gc * CPAD : (gc + 1) * CPAD]
                nc.vector.tensor_scalar(
                    oh, iota_b[:], gtT[:, gtcol(gc) : gtcol(gc) + 1], None, EQ
                )

            def emit_cnt(gc):
                # all accumulating matmuls use start=False onto the memset bank
                # (sums and counts share it, so nobody may zero the region)
                nc.tensor.matmul(
                    out=cnt1,
                    lhsT=stash[:, gc * CPAD : (gc + 1) * CPAD],
                    rhs=ones1[:],
                    start=False,
                    stop=(gc == (NT - 2) * CPT - 1),
                    skip_group_check=True,
                )

            def emit_sum_mm(gc, rhs_ap):
                nc.tensor.matmul(
                    out=sums,
                    lhsT=stash[:, gc * CPAD : (gc + 1) * CPAD],
                    rhs=rhs_ap,
                    start=False,
                    stop=(gc == NCH - 1),
                    skip_group_check=True,
                )

            def emit_transposes(tp, ib, n, base=0):
                for q in range(base, base + n):
                    nc.tensor.transpose(
                        out=tp[:, q * CH : (q + 1) * CH],
                        in_=ib[:, q * CH : (q + 1) * CH],
                        identity=identr[:],
                    )

            with tc.tile_pool(name="psA", bufs=3, space="PSUM") as psA:
                with tc.tile_pool(name="psOhPre", bufs=1, space="PSUM") as psOhPre:
                    # ---- phase 1, software-pipelined 2 tiles deep ----
                    # PE order: T(0) T(1) MM(0)|T(2) MM(1)|T(3) ... so the
                    # sums matmuls of tile t dispatch after the transposes of
                    # tile t+1 and never wait on the Act copy latency
                    pend = []  # [(tile_idx, rhs_tile), ...] awaiting matmuls

                    def flush_one():
                        tt, rr = pend.pop(0)
                        for q in range(CPT):
                            emit_sum_mm(tt * CPT + q, rr[:, q * CH : (q + 1) * CH])

                    for t in range(NT - 1):
                        ib = imgp.tile([P, TB], f32r, tag="img")
                        nc.sync.dma_start(
                            out=ib[:], in_=img.ap()[:, t * TB : (t + 1) * TB]
                        )
                        tp = psA.tile([P, TB], f32r, tag="a")
                        emit_transposes(tp, ib, CPT)
                        if t < NT - 2:
                            for q in range(CPT):
                                emit_onehot(t * CPT + q)
                            # counts only need the one-hot, not the img data:
                            # emit them here so the count chain closes (and rcp
                            # fires) before the sums pipeline drains
                            for q in range(CPT):
                                emit_cnt(t * CPT + q)
                        rhs = rhsp.tile([P, TB], bf16, tag="rhs")
                        nc.scalar.copy(out=rhs[:], in_=tp[:])
                        pend.append((t, rhs))
                        if len(pend) > 2:
                            flush_one()
                        if t < 4:
                            # early one-hots for the last two tiles (spread to
                            # avoid a DVE backlog) -> the count chain closes at
                            # tile NT-3
                            for gc in EARLY_CHUNKS[t * 4 : (t + 1) * 4]:
                                emit_onehot(gc)
                                emit_cnt(gc)
                        if t == NT - 3:
                            nc.vector.tensor_scalar_add(
                                cnt_eps[:], cnt1, EPS
                            )
                            nc.vector.reciprocal(out=rcp[:], in_=cnt_eps[:])
                        if t in (NT - 10, NT - 9, NT - 8):
                            # prebuild ohT tiles for output kilopixels 0..2
                            i = t - (NT - 10)
                            ohp = psOhPre.tile([CPAD, TB], bf16, tag="ohpre")
                            for q in range(CPT):
                                gc = i * CPT + q
                                nc.tensor.transpose(
                                    out=ohp[:, q * CH : (q + 1) * CH],
                                    in_=stash[:, gc * CPAD : (gc + 1) * CPAD],
                                    identity=ident16[:],
                                )
                            pre = ohspre.tile([CPAD, TB], bf16, tag=f"pre{i}")
                            nc.vector.tensor_copy(out=pre[:], in_=ohp[:])
                            if i == 0:
                                pre0 = pre
                            elif i == 1:
                                pre1 = pre
                            else:
                                pre2 = pre

                    # ---- tail tile NT-1 as two 512-px groups ----
                    t = NT - 1
                    ib = imgp.tile([P, TB], f32r, tag="img")
                    nc.sync.dma_start(
                        out=ib[:, 0:640], in_=img.ap()[:, t * TB : t * TB + 640]
                    )
                    nc.sync.dma_start(
                        out=ib[:, 640:TB],
                        in_=img.ap()[:, t * TB + 640 : (t + 1) * TB],
                    )
                    # separate PSUM tiles per tail group: tile-granular dep
                    # tracking must not couple the a- and b-chains
                    tp_a = psA.tile([P, TB], f32r, tag="a")
                    tp_b = psA.tile([P, TB], f32r, tag="a")
                    # both transpose groups up front: T(b) gates the whole
                    # boundary, so it must not queue behind pending matmuls
                    emit_transposes(tp_a, ib, 5, base=0)
                    emit_transposes(tp_b, ib, 3, base=5)
                    # two separate rhs tiles: tile-granular dependency
                    # tracking would otherwise serialize the two tail copies
                    rhs_a = rhsp.tile([P, TB], bf16, tag="rhs")
                    rhs_b = rhsp.tile([P, TB], bf16, tag="rhs")
                    # group-a copy on DVE (ready first), group-b on Act: the
                    # two tail copies run on different engines, and the b-chain
                    # does not queue behind the a-copy
                    nc.vector.tensor_copy(out=rhs_a[:, 0:640], in_=tp_a[:, 0:640])
                    nc.scalar.copy(out=rhs_b[:, 640:TB], in_=tp_b[:, 640:TB])
                    flush_one()
                    flush_one()
                    # emit matmuls in readiness order; the b-group closes the
                    # chain into means
                    for q in range(5):
                        emit_sum_mm(t * CPT + q, rhs_a[:, q * CH : (q + 1) * CH])
                    for q in range(5, CPT):
                        emit_sum_mm(t * CPT + q, rhs_b[:, q * CH : (q + 1) * CH])

            # ---- means (rcp already computed) ----
            nc.vector.tensor_scalar(means[:], sums, rcp[:, 0:1], None, MULT)

            # ---- phase 2: out[ch, px] = means[gt[px], ch] ----
            with (
                tc.tile_pool(name="psOut", bufs=2, space="PSUM") as psOut,
                tc.tile_pool(name="psOh", bufs=3, space="PSUM") as psOh,
            ):
                # group 0 (px 0..1024) split small to restart the DMA stream fast
                op0 = psOut.tile([P, TB], f32, tag="op")
                ob0 = outp.tile([P, TB], f32, tag="ob")
                for i, (s, e) in enumerate(((0, 512), (512, 1024))):
                    nc.tensor.matmul(
                        out=op0[:, s:e], lhsT=means[:], rhs=pre0[:, s:e],
                        start=True, stop=True,
                    )
                    if i % 2 == 0:
                        nc.vector.tensor_copy(out=ob0[:, s:e], in_=op0[:, s:e])
                    else:
                        nc.scalar.copy(out=ob0[:, s:e], in_=op0[:, s:e])
                    nc.sync.dma_start(out=out.ap()[:, s:e], in_=ob0[:, s:e])

                # groups 1..NT-1, software-pipelined one group deep:
                # PE order: T_oh(g+1) then MM(g), so MM never waits the DVE copy
                pend2 = []  # [(g, ohs_tile)]

                def flush_out():
                    g2, ohs2 = pend2.pop(0)
                    op = psOut.tile([P, TB], f32, tag="op")
                    for h in range(2):
                        nc.tensor.matmul(
                            out=op[:, h * 512 : (h + 1) * 512],
                            lhsT=means[:],
                            rhs=ohs2[:, h * 512 : (h + 1) * 512],
                            start=True, stop=True,
                        )
                    ob = outp.tile([P, TB], f32, tag="ob")
                    if g2 % 2 == 0:
                        nc.scalar.copy(out=ob[:], in_=op[:])
                    else:
                        nc.vector.tensor_copy(out=ob[:], in_=op[:])
                    nc.sync.dma_start(
                        out=out.ap()[:, g2 * TB : (g2 + 1) * TB], in_=ob[:]
                    )

                for g in range(1, NT):
                    if g == 1:
                        pend2.append((1, pre1))
                    elif g == 2:
                        pend2.append((2, pre2))
                    else:
                        ohp = psOh.tile([CPAD, TB], bf16, tag="oh")
                        for q in range(CPT):
                            gc = g * CPT + q
                            nc.tensor.transpose(
                                out=ohp[:, q * CH : (q + 1) * CH],
                                in_=stash[:, gc * CPAD : (gc + 1) * CPAD],
                                identity=ident16[:],
                            )
                        ohs = ohsp.tile([CPAD, TB], bf16, tag="oh")
                        if g % 2 == 0:
                            nc.vector.tensor_copy(out=ohs[:], in_=ohp[:])
                        else:
                            nc.scalar.copy(out=ohs[:], in_=ohp[:])
                        pend2.append((g, ohs))

                    if len(pend2) > 2:
                        flush_out()
                while pend2:
                    flush_out()

    nc.compile()
    return nc


def get_module():
    if "nc" not in _CACHE:
        _CACHE["nc"] = _build_module()
    return _CACHE["nc"]


def kernel(img, gt):
    from concourse.bass_utils import run_bass_kernel_spmd

    img = np.asarray(img)
    gt = np.asarray(gt)
    B, C, H, W = img.shape
    assert (B, C, H * W) == (N_CORES, P, HW), (img.shape,)
    img2 = np.ascontiguousarray(img.reshape(B, C, H * W))
    gt2 = np.ascontiguousarray(gt.reshape(B, H * W))

    nc = get_module()
    in_maps = [{"img": img2[i], "gt": gt2[i]} for i in range(B)]
    res = run_bass_kernel_spmd(nc, in_maps, core_ids=list(range(N_CORES)))
    out = np.stack([res.results[i]["out"] for i in range(B)], axis=0)
    return out.reshape(B, C, H, W).astype(np.float32, copy=False)


if __name__ == "__main__":
    # quick self-exercise with random data
    rng = np.random.default_rng(0)
    img = rng.standard_normal((8, 128, 256, 256), dtype=np.float32)
    gt = rng.integers(0, NCLS, size=(8, 1, 256, 256), dtype=np.int32)
    out = kernel(img=img, gt=gt)
    print("out", out.shape, out.dtype)



# revision 9
# speedup vs baseline: 1.3472x; 1.3472x over previous
"""Trainium2 Bass kernel: per-(image, channel) class-mean replacement (segment mean + gather).

Input:  img [8, 128, 256, 256] f32, gt [8, 1, 256, 256] int32 (labels in [0, 21))
Output: out[b, c, h, w] = mean over pixels p of img[b, c, p] where gt[b, p] == gt[b, h, w]

Sharding: data-parallel over batch — each of the 8 NeuronCores processes one image.

Per-core algorithm (C=128 channels on partitions, HW=65536 pixels on free axis):
  Phase 1 (sums):   PE-transpose 128x128 img chunks (f32r stationary, 1.5 cyc/row) to
                    [pix, ch] PSUM; one Act copy per 1024-px tile PSUM->SBUF (bf16);
                    one-hot [128pix, 32cls] bf16 on DVE from gtT; matmul-accumulate
                    sums[32, 128] and counts[32, 1] into a memset-initialized PSUM bank
                    (all matmuls start=False so the two accumulators share one bank).
                    Emission is software-pipelined two tiles deep so the PE never waits
                    on the PSUM->SBUF copy latency.
  Means:            counts only depend on gt, and the one-hots for the last two tiles
                    are emitted early, so rcp = 1/(cnt+eps) is ready before the last
                    img tile lands; means = sums * rcp (bf16) is the only barrier.
  Phase 2 (gather): PE-transpose stash one-hots to [32cls, pix], DVE copy to SBUF,
                    matmul means.T @ onehotT -> out[128ch, pix] PSUM, Act copy to SBUF,
                    DMA out in 1024-px tiles (software-pipelined one tile deep).
  Boundary:         input ends with 640+384-px DMAs processed as short chains on
                    disjoint engines/tiles; ohT tiles for the first 3 output kilopixels
                    are prebuilt during phase 1; first outputs are 256/256/512 px so the
                    output DMA stream restarts quickly after the means barrier.
"""

import os
import sys

for _p in ("/opt/trn_rl_repo", "/root/.axon_site/_ro/trn_rl_repo"):
    if os.path.isdir(_p) and _p not in sys.path:
        sys.path.append(_p)

import numpy as np

P = 128          # channels == SBUF partitions
HW = 256 * 256   # pixels per image
NCLS = 21
CPAD = 32        # padded class count
CH = 128         # pixels per matmul chunk
NCH = HW // CH   # 512 chunks
TB = 1024        # pixels per input tile / rhs copy group
NT = HW // TB    # 64 tiles
CPT = TB // CH   # 8 chunks per tile
EPS = 1e-8
N_CORES = 8

# chunks whose one-hot + count matmuls are emitted early (the last two tiles),
# so the count accumulation chain finishes before the last img DMA lands
EARLY_CHUNKS = list(range((NT - 2) * CPT, NCH))

_CACHE = {}


def _build_module(variant="full"):
    import concourse.bacc as bacc
    import concourse.mybir as mybir
    import concourse.tile as tile
    from concourse.masks import make_identity

    f32 = mybir.dt.float32
    f32r = mybir.dt.float32r
    bf16 = mybir.dt.bfloat16
    i32 = mybir.dt.int32
    EQ = mybir.AluOpType.is_equal
    MULT = mybir.AluOpType.mult

    nc = bacc.Bacc("TRN2", target_bir_lowering=False, debug=False)
    img = nc.dram_tensor("img", [P, HW], bf16, kind="ExternalInput")
    gt = nc.dram_tensor("gt", [HW], i32, kind="ExternalInput")
    out = nc.dram_tensor("out", [P, HW], bf16, kind="ExternalOutput")

    def gtcol(gc):
        # chunk gc's labels live in gtT at col 32*(gc%16) + gc//16
        return 32 * (gc % 16) + gc // 16

    with tile.TileContext(nc) as tc:
        with (
            tc.tile_pool(name="constp", bufs=1) as constp,
            tc.tile_pool(name="imgp", bufs=6) as imgp,
            tc.tile_pool(name="rhsp", bufs=6) as rhsp,
            tc.tile_pool(name="stashp", bufs=1) as stashp,
            tc.tile_pool(name="ohspre", bufs=1) as ohspre,
            tc.tile_pool(name="ohsp", bufs=4) as ohsp,
            tc.tile_pool(name="outp", bufs=5) as outp,
            tc.tile_pool(name="psS", bufs=1, space="PSUM") as psS,
        ):
            # ---- constants ----
            ident32 = constp.tile([P, P], f32, tag="id32")
            make_identity(nc, ident32[:])
            ident16 = constp.tile([P, P], bf16, tag="id16")
            nc.vector.tensor_copy(out=ident16[:], in_=ident32[:])
            identr = constp.tile([P, P], f32r, tag="idr")
            nc.vector.tensor_copy(out=identr[:], in_=ident32[:])
            iota_b = constp.tile([P, CPAD], bf16, tag="iota")
            nc.gpsimd.iota(
                iota_b[:],
                pattern=[[1, CPAD]],
                base=0,
                channel_multiplier=0,
                allow_small_or_imprecise_dtypes=True,
            )
            ones1 = constp.tile([P, 1], bf16, tag="ones1")
            nc.vector.memset(ones1[:], 1.0)

            # ---- gt preprocessing: labels for chunk gc onto partitions ----
            gtn_i = constp.tile([32, HW // 32], i32, tag="gtn_i")
            nc.sync.dma_start(
                out=gtn_i[:], in_=gt.ap().rearrange("(p f) -> p f", p=32)
            )
            gtn_f = constp.tile([32, HW // 32], f32, tag="gtn_f")
            for h in range(2):
                nc.vector.tensor_copy(
                    out=gtn_f[:, h * 1024 : (h + 1) * 1024],
                    in_=gtn_i[:, h * 1024 : (h + 1) * 1024],
                )
            gtT = constp.tile([P, NCH], f32, tag="gtT")
            with tc.tile_pool(name="psGT", bufs=1, space="PSUM") as psGT:
                gps = psGT.tile([P, NCH], f32, tag="gt")
                for b in range(16):
                    nc.tensor.transpose(
                        out=gps[:, b * 32 : (b + 1) * 32],
                        in_=gtn_f[:, b * P : (b + 1) * P],
                        identity=ident32[0:32, 0:32],
                    )
                nc.vector.tensor_copy(out=gtT[:], in_=gps[:])

            # one-hot stash for the whole image: chunk gc at cols [32gc, 32gc+32)
            stash = stashp.tile([P, CPAD * NCH], bf16, tag="stash")
            sums_cnt = psS.tile([CPAD, P + 4], f32, tag="sums")
            nc.vector.memset(sums_cnt[:], 0.0)
            sums = sums_cnt[:, 0:P]
            cnt1 = sums_cnt[:, P : P + 1]
            means = constp.tile([CPAD, P], bf16, tag="means")
            rcp = constp.tile([CPAD, 1], f32, tag="rcp")
            cnt_eps = constp.tile([CPAD, 1], f32, tag="cnt_eps")

            def emit_onehot(gc):
                oh = stash[:, gc * CPAD : (gc + 1) * CPAD]
                nc.vector.tensor_scalar(
                    oh, iota_b[:], gtT[:, gtcol(gc) : gtcol(gc) + 1], None, EQ
                )

            def emit_cnt(gc):
                # all accumulating matmuls use start=False onto the memset bank
                # (sums and counts share it, so nobody may zero the region)
                nc.tensor.matmul(
                    out=cnt1,
                    lhsT=stash[:, gc * CPAD : (gc + 1) * CPAD],
                    rhs=ones1[:],
                    start=False,
                    stop=(gc == (NT - 2) * CPT - 1),
                    skip_group_check=True,
                )

            def emit_sum_mm(gc, rhs_ap):
                nc.tensor.matmul(
                    out=sums,
                    lhsT=stash[:, gc * CPAD : (gc + 1) * CPAD],
                    rhs=rhs_ap,
                    start=False,
                    stop=(gc == NCH - 1),
                    skip_group_check=True,
                )

            def emit_transposes(tp, ib, n, base=0):
                for q in range(base, base + n):
                    nc.tensor.transpose(
                        out=tp[:, q * CH : (q + 1) * CH],
                        in_=ib[:, q * CH : (q + 1) * CH],
                        identity=ident16[:],
                    )

            with tc.tile_pool(name="psA", bufs=3, space="PSUM") as psA:
                with tc.tile_pool(name="psOhPre", bufs=1, space="PSUM") as psOhPre:
                    # ---- phase 1, software-pipelined 2 tiles deep ----
                    # PE order: T(0) T(1) MM(0)|T(2) MM(1)|T(3) ... so the
                    # sums matmuls of tile t dispatch after the transposes of
                    # tile t+1 and never wait on the Act copy latency
                    pend = []  # [(tile_idx, rhs_tile), ...] awaiting matmuls

                    def flush_one():
                        tt, rr = pend.pop(0)
                        for q in range(CPT):
                            emit_sum_mm(tt * CPT + q, rr[:, q * CH : (q + 1) * CH])

                    for t in range(NT - 1):
                        ib = imgp.tile([P, TB], bf16, tag="img")
                        nc.sync.dma_start(
                            out=ib[:], in_=img.ap()[:, t * TB : (t + 1) * TB]
                        )
                        tp = psA.tile([P, TB], bf16, tag="a")
                        emit_transposes(tp, ib, CPT)
                        if t < NT - 2:
                            for q in range(CPT):
                                emit_onehot(t * CPT + q)
                            # counts only need the one-hot, not the img data:
                            # emit them here so the count chain closes (and rcp
                            # fires) before the sums pipeline drains
                            for q in range(CPT):
                                emit_cnt(t * CPT + q)
                        rhs = rhsp.tile([P, TB], bf16, tag="rhs")
                        nc.scalar.copy(out=rhs[:], in_=tp[:])
                        pend.append((t, rhs))
                        if len(pend) > 2:
                            flush_one()
                        if t < 4:
                            # early one-hots for the last two tiles (spread to
                            # avoid a DVE backlog) -> the count chain closes at
                            # tile NT-3
                            for gc in EARLY_CHUNKS[t * 4 : (t + 1) * 4]:
                                emit_onehot(gc)
                                emit_cnt(gc)
                        if t == NT - 3:
                            nc.vector.tensor_scalar_add(
                                cnt_eps[:], cnt1, EPS
                            )
                            nc.vector.reciprocal(out=rcp[:], in_=cnt_eps[:])
                        if t in (NT - 10, NT - 9, NT - 8):
                            # prebuild ohT tiles for output kilopixels 0..2
                            i = t - (NT - 10)
                            ohp = psOhPre.tile([CPAD, TB], bf16, tag="ohpre")
                            for q in range(CPT):
                                gc = i * CPT + q
                                nc.tensor.transpose(
                                    out=ohp[:, q * CH : (q + 1) * CH],
                                    in_=stash[:, gc * CPAD : (gc + 1) * CPAD],
                                    identity=ident16[:],
                                )
                            pre = ohspre.tile([CPAD, TB], bf16, tag=f"pre{i}")
                            nc.vector.tensor_copy(out=pre[:], in_=ohp[:])
                            if i == 0:
                                pre0 = pre
                            elif i == 1:
                                pre1 = pre
                            else:
                                pre2 = pre

                    # ---- tail tile NT-1 as two 512-px groups ----
                    t = NT - 1
                    ib = imgp.tile([P, TB], bf16, tag="img")
                    nc.sync.dma_start(
                        out=ib[:, 0:640], in_=img.ap()[:, t * TB : t * TB + 640]
                    )
                    nc.sync.dma_start(
                        out=ib[:, 640:TB],
                        in_=img.ap()[:, t * TB + 640 : (t + 1) * TB],
                    )
                    # separate PSUM tiles per tail group: tile-granular dep
                    # tracking must not couple the a- and b-chains
                    tp_a = psA.tile([P, TB], bf16, tag="a")
                    tp_b = psA.tile([P, TB], bf16, tag="a")
                    # both transpose groups up front: T(b) gates the whole
                    # boundary, so it must not queue behind pending matmuls
                    emit_transposes(tp_a, ib, 5, base=0)
                    emit_transposes(tp_b, ib, 3, base=5)
                    # two separate rhs tiles: tile-granular dependency
                    # tracking would otherwise serialize the two tail copies
                    rhs_a = rhsp.tile([P, TB], bf16, tag="rhs")
                    rhs_b = rhsp.tile([P, TB], bf16, tag="rhs")
                    # group-a copy on DVE (ready first), group-b on Act: the
                    # two tail copies run on different engines, and the b-chain
                    # does not queue behind the a-copy
                    nc.vector.tensor_copy(out=rhs_a[:, 0:640], in_=tp_a[:, 0:640])
                    nc.scalar.copy(out=rhs_b[:, 640:TB], in_=tp_b[:, 640:TB])
                    flush_one()
                    flush_one()
                    # emit matmuls in readiness order; the b-group closes the
                    # chain into means
                    for q in range(5):
                        emit_sum_mm(t * CPT + q, rhs_a[:, q * CH : (q + 1) * CH])
                    for q in range(5, CPT):
                        emit_sum_mm(t * CPT + q, rhs_b[:, q * CH : (q + 1) * CH])

            # ---- means (rcp already computed) ----
            nc.vector.tensor_scalar(means[:], sums, rcp[:, 0:1], None, MULT)

            # ---- phase 2: out[ch, px] = means[gt[px], ch] ----
            with (
                tc.tile_pool(name="psOut", bufs=2, space="PSUM") as psOut,
                tc.tile_pool(name="psOh", bufs=3, space="PSUM") as psOh,
            ):
                # group 0 (px 0..1024) split small to restart the DMA stream fast
                op0 = psOut.tile([P, TB], f32, tag="op")
                ob0 = outp.tile([P, TB], bf16, tag="ob")
                for i, (s, e) in enumerate(((0, 512), (512, 1024))):
                    nc.tensor.matmul(
                        out=op0[:, s:e], lhsT=means[:], rhs=pre0[:, s:e],
                        start=True, stop=True,
                    )
                    if i % 2 == 0:
                        nc.vector.tensor_copy(out=ob0[:, s:e], in_=op0[:, s:e])
                    else:
                        nc.scalar.copy(out=ob0[:, s:e], in_=op0[:, s:e])
                    nc.sync.dma_start(out=out.ap()[:, s:e], in_=ob0[:, s:e])

                # groups 1..NT-1, software-pipelined one group deep:
                # PE order: T_oh(g+1) then MM(g), so MM never waits the DVE copy
                pend2 = []  # [(g, ohs_tile)]

                def flush_out():
                    g2, ohs2 = pend2.pop(0)
                    op = psOut.tile([P, TB], f32, tag="op")
                    for h in range(2):
                        nc.tensor.matmul(
                            out=op[:, h * 512 : (h + 1) * 512],
                            lhsT=means[:],
                            rhs=ohs2[:, h * 512 : (h + 1) * 512],
                            start=True, stop=True,
                        )
                    ob = outp.tile([P, TB], bf16, tag="ob")
                    if g2 % 2 == 0:
                        nc.scalar.copy(out=ob[:], in_=op[:])
                    else:
                        nc.vector.tensor_copy(out=ob[:], in_=op[:])
                    nc.sync.dma_start(
                        out=out.ap()[:, g2 * TB : (g2 + 1) * TB], in_=ob[:]
                    )

                for g in range(1, NT):
                    if g == 1:
                        pend2.append((1, pre1))
                    elif g == 2:
                        pend2.append((2, pre2))
                    else:
                        ohp = psOh.tile([CPAD, TB], bf16, tag="oh")
                        for q in range(CPT):
                            gc = g * CPT + q
                            nc.tensor.transpose(
                                out=ohp[:, q * CH : (q + 1) * CH],
                                in_=stash[:, gc * CPAD : (gc + 1) * CPAD],
                                identity=ident16[:],
                            )
                        ohs = ohsp.tile([CPAD, TB], bf16, tag="oh")
                        if g % 2 == 0:
                            nc.vector.tensor_copy(out=ohs[:], in_=ohp[:])
                        else:
                            nc.scalar.copy(out=ohs[:], in_=ohp[:])
                        pend2.append((g, ohs))

                    if len(pend2) > 2:
                        flush_out()
                while pend2:
                    flush_out()

    nc.compile()
    return nc


def get_module():
    if "nc" not in _CACHE:
        _CACHE["nc"] = _build_module()
    return _CACHE["nc"]


def kernel(img, gt):
    from concourse.bass_utils import run_bass_kernel_spmd
    from ml_dtypes import bfloat16

    img = np.asarray(img)
    gt = np.asarray(gt)
    B, C, H, W = img.shape
    assert (B, C, H * W) == (N_CORES, P, HW), (img.shape,)
    # device I/O in bf16: halves HBM traffic; rel-err budget (2e-2) covers it
    img2 = np.ascontiguousarray(img.reshape(B, C, H * W)).astype(bfloat16)
    gt2 = np.ascontiguousarray(gt.reshape(B, H * W))

    nc = get_module()
    in_maps = [{"img": img2[i], "gt": gt2[i]} for i in range(B)]
    res = run_bass_kernel_spmd(nc, in_maps, core_ids=list(range(N_CORES)))
    out = np.stack([res.results[i]["out"] for i in range(B)], axis=0)
    return out.reshape(B, C, H, W).astype(np.float32)


if __name__ == "__main__":
    # quick self-exercise with random data
    rng = np.random.default_rng(0)
    img = rng.standard_normal((8, 128, 256, 256), dtype=np.float32)
    gt = rng.integers(0, NCLS, size=(8, 1, 256, 256), dtype=np.int32)
    out = kernel(img=img, gt=gt)
    print("out", out.shape, out.dtype)

